# revision 13
# baseline (speedup 1.0000x reference)
"""CUBA-LIF spiking network (2 synapse layers + readouts) on 8 trn2 NeuronCores.

Sharding: data-parallel over batch (8 samples per core), everything else local.
Per-core pipeline (all fp32):
  GEMM0 (PE)  -> z0 [t-major]   -> current scan (DVE tensor_tensor_scan)
  voltage loop: 2 scalar_tensor_tensor ops per timestep (DVE)
  spike extraction: vectorized is_ge post-pass -> GEMM-ready layout
  GEMM1 (PE, w1T streamed per output block) -> same scan/loop
  readout GEMMs (PE)
Host side only reshapes/transposes numpy arrays.
"""

import numpy as np

B, I, H, O, T = 64, 700, 2048, 20, 100
IP = 768           # I padded to 6*128
NC = 8             # cores
BL = B // NC       # 8 samples per core
KI = IP // 128     # 6  k-tiles for layer 0
KH = H // 128      # 16 k-tiles for layer 1 / readout
MB = H // 128      # 16 output blocks per layer
TH = T // 2        # 50: t-half per matmul n-chunk (n = 50*8 = 400)

THR = 1.25
CD, VD = 0.25, 0.03

_CACHE = {}


def _build():
    import concourse.bass as bass
    import concourse.bacc as bacc
    import concourse.tile as tile
    from concourse import mybir
    from contextlib import ExitStack

    f32 = mybir.dt.float32
    Alu = mybir.AluOpType

    # Bacc (not raw Bass): its compile() runs move_matmul_waits_to_ldweights
    # and generate_event_semaphores, which legalize multi-wait instructions
    # for walrus (PE/DMA structs only hold one plain sem wait).
    nc = bacc.Bacc()
    spike_in = nc.declare_dram_parameter("spike", [128, KI, T, BL], f32, isOutput=False)
    w0t_in = nc.declare_dram_parameter("w0t", [MB, 128, KI, 128], f32, isOutput=False)
    w1t_in = nc.declare_dram_parameter("w1t", [MB, 128, KH, 128], f32, isOutput=False)
    rot_in = nc.declare_dram_parameter("rot", [128, 2, KH, O], f32, isOutput=False)
    out_v = nc.declare_dram_parameter("v_out", [2, 128, T, 128], f32, isOutput=True)
    out_s = nc.declare_dram_parameter("s_out", [2, 128, KH, T, BL], f32, isOutput=True)
    out_r = nc.declare_dram_parameter("r_out", [2, O, T, BL], f32, isOutput=True)

    with ExitStack() as ctx:
        tc = ctx.enter_context(tile.TileContext(nc))
        const = ctx.enter_context(tc.tile_pool(name="const", bufs=1))
        big = ctx.enter_context(tc.tile_pool(name="big", bufs=1))
        statep = ctx.enter_context(tc.tile_pool(name="statep", bufs=1))
        wpool = ctx.enter_context(tc.tile_pool(name="wpool", bufs=3))
        rpool = ctx.enter_context(tc.tile_pool(name="rpool", bufs=2))
        psum = ctx.enter_context(tc.tile_pool(name="psum", bufs=4, space="PSUM"))
        rpsum = ctx.enter_context(tc.tile_pool(name="rpsum", bufs=2, space="PSUM"))
        spsum = ctx.enter_context(tc.tile_pool(name="spsum", bufs=1, space="PSUM"))

        # The walrus Matmult lowering only supports ONE semaphore wait per
        # instruction.  `observe(ap)` issues a 1-column dummy matmul that
        # reads `ap` so the PE's vector clock sees that producer; real
        # matmuls after it then need no wait for that dependency.
        scr = spsum.tile([1, 1], f32, tag="scr")

        def observe(ap):
            nc.tensor.matmul(scr, lhsT=ap, rhs=ap, start=True, stop=True)

        # persistent buffers (reused across layers; Tile orders via WAR/RAW)
        zv = big.tile([128, T, 128], f32, tag="zv")       # z then v; col j=m*8+b
        cur = big.tile([128, 128, T], f32, tag="cur")     # per-j current, t contig
        ss = big.tile([128, KH, T, BL], f32, tag="ss")    # spikes, GEMM-ready
        spk = big.tile([128, KI, T, BL], f32, tag="spk")  # input spikes
        rot_sb = const.tile([128, 2, KH, O], f32)
        c075 = const.tile([128, T], f32)

        nc.vector.memset(c075, 1.0 - CD)
        for k in range(KI):
            nc.gpsimd.dma_start(out=spk[:, k], in_=spike_in[:, k])
        nc.gpsimd.dma_start(out=rot_sb, in_=rot_in[:])
        for k in range(KI):
            observe(spk[:, k, 0, 0:1])

        for l in range(2):
            K = KI if l == 0 else KH
            w_in = w0t_in if l == 0 else w1t_in
            rhs_t = spk if l == 0 else ss

            vr = statep.tile([128, 128], f32, tag="vr")
            nc.vector.memset(vr, 0.0)

            for m in range(MB):
                wt = wpool.tile([128, K, 128], f32, tag="wt")
                for k in range(K):
                    nc.gpsimd.dma_start(out=wt[:, k], in_=w_in[m, :, k])
                observe(wt[:, 0, 0:1])
                for c in range(2):
                    ps = psum.tile([128, TH, BL], f32, tag="ps")
                    for k in range(K):
                        nc.tensor.matmul(
                            ps,
                            lhsT=wt[:, k, :],
                            rhs=rhs_t[:, k, c * TH:(c + 1) * TH, :],
                            start=(k == 0),
                            stop=(k == K - 1),
                        )
                    nc.scalar.copy(
                        out=zv[:, c * TH:(c + 1) * TH, m * BL:(m + 1) * BL],
                        in_=ps,
                    )
                # current recurrence for this block's 8 columns
                for b in range(BL):
                    j = m * BL + b
                    nc.vector.tensor_tensor_scan(
                        out=cur[:, j, :],
                        data0=c075,
                        data1=zv[:, :, j],
                        initial=0.0,
                        op0=Alu.mult,
                        op1=Alu.add,
                    )

            # voltage loop: v_t = (1-vd)*vr + cur_t ; vr = v_t * (v_t < thr)
            for t in range(T):
                nc.vector.scalar_tensor_tensor(
                    out=zv[:, t, :],
                    in0=vr,
                    scalar=1.0 - VD,
                    in1=cur[:, :, t],
                    op0=Alu.mult,
                    op1=Alu.add,
                )
                nc.vector.scalar_tensor_tensor(
                    out=vr,
                    in0=zv[:, t, :],
                    scalar=THR,
                    in1=zv[:, t, :],
                    op0=Alu.is_lt,
                    op1=Alu.mult,
                )

            # spikes: s = (v >= thr), strided read -> GEMM-ready layout
            for k in range(KH):
                nc.vector.tensor_scalar(
                    ss[:, k],
                    zv[:, :, k * BL:(k + 1) * BL],
                    THR,
                    None,
                    Alu.is_ge,
                )

            # let the PE observe the last s-pass write before GEMM1/readout
            observe(ss[:, KH - 1, 0, 0:1])

            # voltages out (t-halves so DMA can start early)
            for c in range(2):
                nc.gpsimd.dma_start(
                    out=out_v[l, :, c * TH:(c + 1) * TH, :],
                    in_=zv[:, c * TH:(c + 1) * TH, :],
                )
            nc.gpsimd.dma_start(out=out_s[l], in_=ss)

            # readout GEMM for this layer
            rsb = rpool.tile([O, T, BL], f32, tag="rsb")
            for c in range(2):
                rp = rpsum.tile([O, TH, BL], f32, tag="rp")
                for k in range(KH):
                    nc.tensor.matmul(
                        rp,
                        lhsT=rot_sb[:, l, k, :],
                        rhs=ss[:, k, c * TH:(c + 1) * TH, :],
                        start=(k == 0),
                        stop=(k == KH - 1),
                    )
                nc.scalar.copy(out=rsb[:, c * TH:(c + 1) * TH, :], in_=rp)
            nc.gpsimd.dma_start(out=out_r[l], in_=rsb)

    nc.finalize()
    return nc


def _prep_host(spike, w0, w1, ro0, ro1):
    w0p = np.zeros((H, IP), np.float32)
    w0p[:, :I] = w0
    w0t = np.ascontiguousarray(
        w0p.reshape(MB, 128, KI, 128).transpose(0, 3, 2, 1))
    w1t = np.ascontiguousarray(
        w1.reshape(MB, 128, KH, 128).transpose(0, 3, 2, 1))
    # rot[p, l, k, o] = ro_l[o, k*128+p]  (partition-major for the DMA)
    rot = np.ascontiguousarray(np.stack(
        [r.reshape(O, KH, 128).transpose(2, 1, 0) for r in (ro0, ro1)], axis=1))

    spikes_core = []
    for c in range(NC):
        sp = np.zeros((BL, IP, T), np.float32)
        sp[:, :I] = spike[c * BL:(c + 1) * BL]
        spikes_core.append(np.ascontiguousarray(
            sp.reshape(BL, KI, 128, T).transpose(2, 1, 3, 0)))
    return spikes_core, w0t, w1t, rot


def kernel(spike, w0, w1, ro0, ro1):
    from concourse.bass_utils import run_bass_kernel_spmd

    spike = np.asarray(spike, np.float32)
    w0 = np.asarray(w0, np.float32)
    w1 = np.asarray(w1, np.float32)
    ro0 = np.asarray(ro0, np.float32)
    ro1 = np.asarray(ro1, np.float32)

    if "nc" not in _CACHE:
        _CACHE["nc"] = _build()
    nc = _CACHE["nc"]

    spikes_core, w0t, w1t, rot = _prep_host(spike, w0, w1, ro0, ro1)
    in_maps = [
        {"spike": spikes_core[c], "w0t": w0t, "w1t": w1t, "rot": rot}
        for c in range(NC)
    ]
    res = run_bass_kernel_spmd(nc, in_maps, core_ids=list(range(NC))).results

    spikes = np.empty((2, B, H, T), np.float32)
    volts = np.empty((2, B, H, T), np.float32)
    reads = np.empty((2, B, O, T), np.float32)
    for c in range(NC):
        bsl = slice(c * BL, (c + 1) * BL)
        v = res[c]["v_out"].reshape(2, 128, T, MB, BL)
        volts[:, bsl] = v.transpose(0, 4, 3, 1, 2).reshape(2, BL, H, T)
        s = res[c]["s_out"].reshape(2, 128, KH, T, BL)
        spikes[:, bsl] = s.transpose(0, 4, 2, 1, 3).reshape(2, BL, H, T)
        r = res[c]["r_out"].reshape(2, O, T, BL)
        reads[:, bsl] = r.transpose(0, 3, 1, 2)
    return spikes, reads, volts
